# revision 4
# baseline (speedup 1.0000x reference)
"""GCL (GNN message-passing) Trainium2 Bass kernel on 8 NeuronCores.

Sharding: edges sorted by destination on host and sharded by destination-node
range (1250 nodes/core) -> each core owns the full segment-sum for its nodes,
no collectives. Node features and weights replicated.

Per core, the edge-MLP first layer is split: e1 = A[row] + Bc[col] where
A = h@we1_top + be1 (per-window SBUF bf16 table, injected via one-hot
matmul) and Bc[col] = we1_bot^T @ h[col]: per-edge h rows are fetched by a
DRAM-source dma_gather in transpose mode (8192-idx calls on 4 SWDGE queues,
landing [D, e] in SBUF) and injected with we1_bot as the stationary operand.

Edges are processed in 1024-edge pairs (two 512-edge macros sharing PSUM
tiles so both silus run 1024 wide):
  e1T[D,1024] = we1_bot @ hT_gathered + A @ S_T      (PSUM, 2x2 matmuls)
  where S_T[n,e] one-hot comes from a host-precomputed u8 table via a
  single DVE is_equal-with-0 op per pair (same for the scatter one-hot)
  e2[e,D] = silu(e1) @ we2 + be2                     (8x128 + K=1 matmul)
  aggT[D,n] += e2^T-scatter via lhsT=e2s, rhs=S      (per-window PSUM)
Node MLP + residual per 128-node window, fp32.
"""
import sys
sys.path.insert(0, '/opt/trn_rl_repo')
import numpy as np
import ml_dtypes

N_NODES = 10000
N_EDGES = 640000
D = 128
NORM = 100.0
NCORES = 8
NPC = N_NODES // NCORES          # 1250 destination nodes per core
NWIN = 10                        # 128-node windows per core
MACRO = 512
PAIR = 1024
CALL = 8192                      # edges per dma_gather call (16 macros)
PAD_ROWLOCAL = 200
NROWS = 10240                    # h table rows (padded)

BF16 = ml_dtypes.bfloat16
_prog_cache = {}


def _wrap_idx16(idx):
    """[n] int -> [128, n/16] int16 wrapped (pos i -> partition i%16, col
    i//16) and replicated into all eight 16-partition groups."""
    n = idx.shape[0]
    block = idx.astype(np.int16).reshape(n // 16, 16).T
    return np.tile(block, (8, 1))


def _build_program(cw_per_window, no_gather=False, no_compute=False):
    import concourse.bacc as bacc
    import concourse.mybir as mybir
    from concourse import tile

    dt = mybir.dt
    AF = mybir.ActivationFunctionType
    ALU = mybir.AluOpType

    nm_w = list(cw_per_window)       # macros (512 edges) per window
    NM = sum(nm_w)
    assert NM % 16 == 0
    NP = NM // 2                     # pairs
    NCALLS = NM // 16
    TOT16 = NM * MACRO // 16

    nc = bacc.Bacc("TRN2", target_bir_lowering=False, debug=False,
                   num_devices=NCORES, num_swdge_queues=4)

    f32, bf16, i16, u8 = dt.float32, dt.bfloat16, dt.int16, dt.uint8
    din = lambda n, s, d=f32: nc.dram_tensor(n, s, d, kind="ExternalInput")
    h_bf_d = din("h_bf", [NROWS, 128], bf16)          # gather table
    hTs_bf_d = din("hTs_bf", [128, NWIN * 128], bf16)
    h_slice = din("h_slice", [NWIN, 128, 128])
    we1_top_bf = din("we1_top_bf", [128, 128], bf16)
    we1_bot_bf = din("we1_bot_bf", [128, 128], bf16)
    be1_bf_d = din("be1_bf", [1, 128], bf16)
    we2b_d = din("we2_bf", [128, 128], bf16)
    be2rep8_bf = din("be2rep8_bf", [1, 1024], bf16)
    wn1_lo_bf = din("wn1_lo_bf", [128, 128], bf16)
    wn1_hi_bf = din("wn1_hi_bf", [128, 128], bf16)
    bn1_col = din("bn1_col", [128, 1])
    wn2_bf_d = din("wn2_bf", [128, 128], bf16)
    bn2_bf_d = din("bn2_bf", [1, 128], bf16)
    ones_bf_d = din("ones_bf", [1, 128], bf16)
    norm_col_d = din("norm_col", [128, 1])
    colidx_d = din("colidx", [128, TOT16], i16)
    st_u8_d = din("st_u8", [128, NM * MACRO], u8)     # (rl(c)-p)%256
    s4_u8_d = din("s4_u8", [128, NM * MACRO], u8)     # (rl(chunk)-j)%256
    out_d = nc.dram_tensor("out", [NWIN, 128, 128], f32, kind="ExternalOutput")

    # window of each macro
    win_of = []
    for w in range(NWIN):
        win_of += [w] * nm_w[w]
    win_of += [NWIN - 1] * (NM - len(win_of))

    with tile.TileContext(nc) as tc:
        with (
            tc.tile_pool(name="persist", bufs=1) as pp,
            tc.tile_pool(name="work", bufs=4) as wp,
            tc.tile_pool(name="upool", bufs=3) as up,
            tc.tile_pool(name="gout", bufs=4) as gp,
            tc.tile_pool(name="ps", bufs=3, space="PSUM") as psp,
            tc.tile_pool(name="psa", bufs=2, space="PSUM") as pspa,
        ):
            def load(t_dram, shape, dtype=f32):
                t = pp.tile(shape, dtype, tag=t_dram.name)
                nc.sync.dma_start(t[:], t_dram.ap())
                return t

            hTs_t = load(hTs_bf_d, [128, NWIN * 128], bf16)
            colidx_t = load(colidx_d, [128, TOT16], i16)
            w1t = load(we1_top_bf, [128, 128], bf16)
            w1b = load(we1_bot_bf, [128, 128], bf16)
            be1r = load(be1_bf_d, [1, 128], bf16)
            w2b = load(we2b_d, [128, 128], bf16)
            be2r = load(be2rep8_bf, [1, 1024], bf16)
            wn1l = load(wn1_lo_bf, [128, 128], bf16)
            wn1h = load(wn1_hi_bf, [128, 128], bf16)
            bn1c = load(bn1_col, [128, 1])
            wn2t = load(wn2_bf_d, [128, 128], bf16)
            bn2r = load(bn2_bf_d, [1, 128], bf16)
            onesb = load(ones_bf_d, [1, 128], bf16)
            norm_col = load(norm_col_d, [128, 1])
            hsl_t = pp.tile([128, NWIN, 128], f32, tag="h_slice")
            nc.sync.dma_start(hsl_t[:], h_slice.ap().rearrange("w p d -> p w d"))

            # ---- A table: h @ we1_top + be1, bf16, SBUF resident ----
            a_bf = pp.tile([128, NWIN, 128], bf16, tag="a_bf")
            for w in range(NWIN):
                ap_ = psp.tile([128, 128], f32, tag="ps")
                nc.tensor.matmul(ap_[:], onesb[:], be1r[:], start=True, stop=False)
                nc.tensor.matmul(ap_[:], hTs_t[:, w * 128:(w + 1) * 128], w1t[:],
                                 start=False, stop=True)
                nc.scalar.activation(a_bf[:, w, :], ap_[:], AF.Copy)

            agg_sb = None
            if not no_compute:
                agg_sb = pp.tile([128, NWIN, 128], bf16, tag="aggsb")

            gts = {}
            uts = {}
            stash = {}
            agg_tiles = {}
            PREF = 3

            def issue_gather(cc):
                if cc >= NCALLS:
                    return
                gt = gp.tile([128, 1, CALL], bf16, tag="g")
                if not no_gather:
                    nc.gpsimd.dma_gather(
                        gt[:], h_bf_d.ap(),
                        colidx_t[:, cc * CALL // 16:(cc + 1) * CALL // 16],
                        num_idxs=CALL, num_idxs_reg=CALL, elem_size=128,
                        transpose=True, single_packet=False,
                        queue_num=cc % 4,
                    )
                else:
                    nc.vector.tensor_copy(gt[:, 0, 0:8], w1b[:, 0:8])
                gts[cc] = gt

            def issue_u8(j):
                """DMA the st/s4 u8 tables for pair j."""
                if j >= NP or no_compute:
                    return
                stt = up.tile([128, PAIR], u8, tag="st8")
                nc.sync.dma_start(stt[:], st_u8_d.ap()[:, j * PAIR:(j + 1) * PAIR])
                s4t = up.tile([128, PAIR], u8, tag="s48")
                nc.sync.dma_start(s4t[:], s4_u8_d.ap()[:, j * PAIR:(j + 1) * PAIR])
                uts[j] = (stt, s4t)

            for p in range(PREF):
                issue_gather(p)
            for p in range(3):
                issue_u8(p)

            def front(j):
                """pair j: one-hot st, e1 matmuls, silu -> e1s."""
                ci = j // 8
                sl = j % 8
                if sl == 0:
                    issue_gather(ci + PREF)
                issue_u8(j + 3)
                gt = gts[ci]
                if no_compute:
                    if sl == 0:
                        sink = wp.tile([128, 8], bf16, tag="sink")
                        nc.vector.tensor_copy(sink[:], gt[:, 0, 0:8])
                    return
                stt, s4t = uts[j]
                st = wp.tile([128, PAIR], bf16, tag="st")
                nc.vector.tensor_scalar(st[:], stt[:], 0.0, None, ALU.is_equal)
                e1p = psp.tile([128, PAIR], f32, tag="ps")
                for h in range(2):
                    m = 2 * j + h
                    o = h * MACRO
                    nc.tensor.matmul(e1p[:, o:o + MACRO], w1b[:],
                                     gt[:, 0, sl * PAIR + o:sl * PAIR + o + MACRO],
                                     start=True, stop=False, skip_group_check=True)
                    nc.tensor.matmul(e1p[:, o:o + MACRO], a_bf[:, win_of[m], :],
                                     st[:, o:o + MACRO],
                                     start=False, stop=True, skip_group_check=True)
                e1s = wp.tile([128, PAIR], bf16, tag="e1s")
                nc.scalar.activation(e1s[:], e1p[:], AF.Silu)
                stash[("e1s", j)] = e1s
                s4 = wp.tile([128, PAIR], bf16, tag="s4")
                nc.vector.tensor_scalar(s4[:], s4t[:], 0.0, None, ALU.is_equal)
                stash[("s4", j)] = s4

            def mid(j):
                """pair j: e2 matmuls + silu -> e2s."""
                if no_compute:
                    return
                e1s = stash.pop(("e1s", j))
                e2p = psp.tile([128, PAIR], f32, tag="ps")
                for h in range(2):
                    nc.tensor.matmul(e2p[:, h * 512:(h + 1) * 512], onesb[:],
                                     be2r[:, h * 512:(h + 1) * 512],
                                     start=True, stop=False,
                                     skip_group_check=True)
                for t in range(8):
                    nc.tensor.matmul(
                        e2p[:, t * 128:(t + 1) * 128],
                        e1s[:, t * 128:(t + 1) * 128], w2b[:],
                        start=False, stop=True, skip_group_check=True)
                e2s = wp.tile([128, PAIR], bf16, tag="e2s")
                nc.scalar.activation(e2s[:], e2p[:], AF.Silu)
                stash[("e2s", j)] = e2s

            def back(j):
                """pair j: scatter into window agg, finalize windows."""
                if no_compute:
                    return
                e2s = stash.pop(("e2s", j))
                s4 = stash.pop(("s4", j))
                for h in range(2):
                    m = 2 * j + h
                    w = win_of[m]
                    first = (m == 0 or win_of[m - 1] != w)
                    last = (m == NM - 1 or win_of[m + 1] != w)
                    if first:
                        agg_new = pspa.tile([128, 128], f32, tag="agg")
                        agg_tiles[w] = agg_new
                    agg_ps = agg_tiles[w]
                    for t in range(4):
                        c = h * MACRO + t * 128
                        nc.tensor.matmul(
                            agg_ps[:], e2s[:, c:c + 128], s4[:, c:c + 128],
                            start=(first and t == 0), stop=(last and t == 3),
                            skip_group_check=True)
                    if last:
                        nc.vector.tensor_scalar(agg_sb[:, w, :], agg_ps[:],
                                                norm_col[:, 0:1], None, ALU.mult)
                        node_phase(w)

            def node_phase(w):
                hp = psp.tile([128, 128], f32, tag="ps")
                nc.tensor.matmul(hp[:], wn1l[:], hTs_t[:, w * 128:(w + 1) * 128],
                                 start=True, stop=False)
                nc.tensor.matmul(hp[:], wn1h[:], agg_sb[:, w, :],
                                 start=False, stop=True)
                hs = wp.tile([128, 128], bf16, tag="hs")
                nc.scalar.activation(hs[:], hp[:], AF.Silu, bias=bn1c[:, 0:1])
                op = psp.tile([128, 128], f32, tag="ps")
                nc.tensor.matmul(op[:], onesb[:], bn2r[:], start=True, stop=False)
                nc.tensor.matmul(op[:], hs[:], wn2t[:], start=False, stop=True)
                ot = wp.tile([128, 128], f32, tag="ot")
                nc.vector.tensor_tensor(ot[:], op[:], hsl_t[:, w, :], ALU.add)
                nc.sync.dma_start(out_d.ap()[w], ot[:])

            for j in range(NP + 2):
                if j < NP:
                    front(j)
                if 1 <= j <= NP:
                    mid(j - 1)
                if j >= 2:
                    back(j - 2)

            if no_compute:
                for w in range(NWIN):
                    nc.sync.dma_start(out_d.ap()[w], hsl_t[:, w, :])

    nc.compile()
    return nc


def _prep_inputs(h, edge_index, we1, be1, we2, be2, wn1, bn1, wn2, bn2):
    """Host-side shard/sort/pad. Returns (cw_per_window, per-core in_maps)."""
    h = np.asarray(h, np.float32)
    row = np.asarray(edge_index[0], np.int64).astype(np.int32)
    col = np.asarray(edge_index[1], np.int64).astype(np.int32)

    # per (core, window) edge lists
    core = row // NPC
    rl_g = row - core * NPC
    win = rl_g // 128
    rl = rl_g % 128

    counts = np.zeros((NCORES, NWIN), np.int64)
    per = [[None] * NWIN for _ in range(NCORES)]
    for cid in range(NCORES):
        msk = core == cid
        w_c, rl_c, col_c = win[msk], rl[msk], col[msk]
        for w in range(NWIN):
            wm = w_c == w
            per[cid][w] = (col_c[wm], rl_c[wm])
            counts[cid, w] = wm.sum()
    cw = [int(-(-counts[:, w].max() // MACRO)) for w in range(NWIN)]
    # pad total macro count to a multiple of 16 (one 8192 gather call each)
    pad = (-sum(cw)) % 16
    cw[-1] += pad
    cw_per_window = tuple(cw)

    nm_w = list(cw_per_window)
    NM = sum(nm_w)

    h_pad = np.zeros((NROWS, 128), np.float32)
    h_pad[:N_NODES] = h
    hT_pad = h_pad.T
    shared = {
        "h_bf": h_pad.astype(BF16),
        "we1_top_bf": np.asarray(we1[:128], np.float32).astype(BF16),
        "we1_bot_bf": np.asarray(we1[128:], np.float32).astype(BF16),
        "be1_bf": np.asarray(be1, np.float32)[None, :].astype(BF16),
        "be2rep8_bf": np.tile(np.asarray(be2, np.float32), 8)[None, :].astype(BF16),
        "wn1_lo_bf": np.asarray(wn1[:128], np.float32).astype(BF16),
        "wn1_hi_bf": np.asarray(wn1[128:], np.float32).astype(BF16),
        "bn1_col": np.asarray(bn1, np.float32)[:, None].copy(),
        "wn2_bf": np.asarray(wn2, np.float32).astype(BF16),
        "bn2_bf": np.asarray(bn2, np.float32)[None, :].astype(BF16),
        "ones_bf": np.ones((1, 128), np.float32).astype(BF16),
        "norm_col": np.full((128, 1), 1.0 / NORM, np.float32),
        "we2_bf": np.asarray(we2, np.float32).astype(BF16),
    }

    p_arr = np.arange(128, dtype=np.int32)
    j_arr = np.tile(np.arange(128, dtype=np.int32), 8)   # c%128 over 1024

    in_maps = []
    for cid in range(NCORES):
        col_all = np.zeros(NM * MACRO, np.int32)
        rl_all = np.full(NM * MACRO, PAD_ROWLOCAL, np.int32)
        pos = 0
        for w in range(NWIN):
            ccol, crl = per[cid][w]
            col_all[pos:pos + len(ccol)] = ccol
            rl_all[pos:pos + len(crl)] = crl
            pos += nm_w[w] * MACRO
        colidx = np.zeros((128, NM * MACRO // 16), np.int16)
        for cc in range(NM // 16):
            colidx[:, cc * CALL // 16:(cc + 1) * CALL // 16] = _wrap_idx16(
                col_all[cc * CALL:(cc + 1) * CALL])
        # st_u8[p, c] = (rl(c) - p) mod 256 ; is_equal 0 -> onehot [node, e]
        st_u8 = ((rl_all[None, :] - p_arr[:, None]) % 256).astype(np.uint8)
        # s4_u8[p, chunk*128+jj] = (rl(chunk*128+p) - jj) mod 256
        rl_chunk = rl_all.reshape(NM * MACRO // 128, 128).T  # [p, chunk]
        s4_u8 = np.empty((128, NM * MACRO), np.int32)
        s4_u8 = (np.repeat(rl_chunk, 128, axis=1)
                 - np.tile(j_arr[:128], NM * MACRO // 128)[None, :]) % 256
        s4_u8 = s4_u8.astype(np.uint8)
        base = cid * NPC
        hTs_bf = np.zeros((128, NWIN * 128), np.float32)
        hTs_bf[:, :] = hT_pad[:, base:base + NWIN * 128]
        h_slice = np.zeros((NWIN, 128, 128), np.float32)
        hi = min(N_NODES, base + NWIN * 128)
        h_slice.reshape(NWIN * 128, 128)[:hi - base] = h[base:hi]
        in_maps.append({**shared, "hTs_bf": hTs_bf.astype(BF16),
                        "h_slice": h_slice, "colidx": colidx,
                        "st_u8": st_u8, "s4_u8": s4_u8})
    return cw_per_window, in_maps


def kernel(**inputs):
    from concourse.bass_utils import run_bass_kernel_spmd

    cw, in_maps = _prep_inputs(**inputs)
    if cw not in _prog_cache:
        _prog_cache[cw] = _build_program(cw)
    nc = _prog_cache[cw]
    res = run_bass_kernel_spmd(nc, in_maps, list(range(NCORES)))
    outs = []
    for cid in range(NCORES):
        o = res.results[cid]["out"].reshape(NWIN * 128, 128)
        outs.append(o[:NPC])
    return np.concatenate(outs, axis=0)[:N_NODES].astype(np.float32)
